# revision 48
# baseline (speedup 1.0000x reference)
"""Causal multi-head attention block (B=16, T=1024, C=768, H=12) on 8 Trainium2
NeuronCores.

Strategy: data-parallel over batch (2 batches per core, no collectives).
Per-core pipeline, all matmul operands bf16 with fp32 PSUM accumulation:
  A(b) qk^T = W_qk^T x^T (features on partitions); v = x W_v natural with a
       ones column per head (softmax denominators for free)
  B(b) per head-pair: S^T = K Q^T packed two heads per PE pass (K=64
       row-tiling, head B's output shifted down so exp covers one contiguous
       range), exp on ACT, causal mask via GPSIMD with the masked tiles'
       PV matmuls deferred to the unit tail (mask latency off the PE path)
  N(b) per unit: DVE reciprocal reads the denominator rows straight from
       PSUM, DRAM bounce broadcasts them, one [128,NI] DVE multiply
       normalizes both heads; the multiply is emitted one unit late so the
       DVE queue never waits on the broadcast DMA
  P(b) out = y W_proj + b_proj  (y^T is exactly the stationary operand layout)

Schedule: 8 preheat matmuls on zeroed tiles warm the PE HAM clock gate while
the first DMAs land; A(0) head, then B(0) with A(1) woven between units, then
B(1) (i-block 1 first) with P(0)/P(1) fillers; the final projections contract
the last-finished head-pair last so only a short normalize chain trails.
1/sqrt(64) folded into W_q host-side; biases applied via DVE epilogues.
"""

import sys

sys.path.insert(0, "/opt/trn_rl_repo")

import numpy as np
import ml_dtypes

import concourse.bass as bass
import concourse.tile as tile
from concourse import mybir
from concourse.bass_utils import run_bass_kernel_spmd

F32 = mybir.dt.float32
BF16 = mybir.dt.bfloat16

N_CORES = 8
B, T, C = 16, 1024, 768
H, DH = 12, 64
NB = B // N_CORES          # local batches per core (2)
R = NB * T                 # local rows (2048)
KT = C // 128              # contraction tiles (6)
FT = (2 * C) // 128        # qk feature tiles (12)
HP = H // 2                # head pairs (6)
NI = 512                   # i-block width
NIB = T // NI              # i-blocks per batch (2)
JT = T // 128              # j tiles per batch (8)


def _split_excess_waits(nc):
    """Walrus for this target accepts 1 semaphore wait per instruction
    (2 for EventSemaphore). Tile can emit more; split the excess onto
    same-engine nops placed immediately before the instruction."""
    nsplit = 0
    fn = nc.m.functions[0]
    cur = nc.cur_bb.bb if hasattr(nc.cur_bb, "bb") else nc.cur_bb
    for blk in fn.blocks:
        insts = list(blk.instructions)
        if not any(
            i.sync_info is not None
            and i.sync_info.on_wait
            and len(i.sync_info.on_wait)
            > (2 if type(i).__name__ == "InstEventSemaphore" else 1)
            for i in insts
        ):
            continue
        newlist, made = [], []
        for inst in insts:
            si = inst.sync_info
            maxw = 2 if type(inst).__name__ == "InstEventSemaphore" else 1
            if si is not None and si.on_wait and len(si.on_wait) > maxw:
                waits = list(si.on_wait)
                extra, keep = waits[:-maxw], waits[-maxw:]
                si.on_wait = keep
                for w in extra:
                    nop = nc.engines[inst.engine].nop()
                    nop.ins.sync_info = mybir.SyncInfo(on_wait=[w], on_update=[])
                    made.append(nop.ins)
                    newlist.append(nop.ins)
                    nsplit += 1
            newlist.append(inst)
        for m in made:
            if m in cur.instructions:
                cur.instructions.remove(m)
        blk.instructions[:] = newlist
    return nsplit


def _build_program():
    from contextlib import ExitStack

    nc = bass.Bass("TRN2", target_bir_lowering=False, debug=False)

    xs_d = nc.dram_tensor("xs", [C, R], BF16, kind="ExternalInput").ap()
    wqk_d = nc.dram_tensor("wqk", [C, 2 * C], BF16, kind="ExternalInput").ap()
    wv_d = nc.dram_tensor("wv", [C, C], BF16, kind="ExternalInput").ap()
    wp_d = nc.dram_tensor("wp", [C, C], BF16, kind="ExternalInput").ap()
    bqk_d = nc.dram_tensor("bqk", [2 * C], F32, kind="ExternalInput").ap()
    bv_d = nc.dram_tensor("bv", [C], F32, kind="ExternalInput").ap()
    bp_d = nc.dram_tensor("bp", [C], F32, kind="ExternalInput").ap()
    cm_d = nc.dram_tensor("cmask", [128, 128], BF16, kind="ExternalInput").ap()
    out_d = nc.dram_tensor("out", [R, C], F32, kind="ExternalOutput").ap()

    with tile.TileContext(nc) as tc, ExitStack() as ctx:
        persist = ctx.enter_context(tc.tile_pool(name="persist", bufs=1))
        work = ctx.enter_context(tc.tile_pool(name="work", bufs=4))
        pT_pool = ctx.enter_context(tc.tile_pool(name="pTp", bufs=5))
        bc_pool = ctx.enter_context(tc.tile_pool(name="bcp", bufs=3))
        den_pool = ctx.enter_context(tc.tile_pool(name="denp", bufs=3))
        dc_pool = ctx.enter_context(tc.tile_pool(name="dcp", bufs=4))
        ps01 = ctx.enter_context(tc.tile_pool(name="ps01", bufs=2, space="PSUM"))
        psS = ctx.enter_context(tc.tile_pool(name="psS", bufs=2, space="PSUM"))
        psPV = ctx.enter_context(tc.tile_pool(name="psPV", bufs=2, space="PSUM"))
        dpool = ctx.enter_context(tc.tile_pool(name="dpool", bufs=4, space="DRAM"))

        wqk = persist.tile([128, KT, 2 * C], BF16)
        wv = persist.tile([128, KT, C], BF16)
        wp = persist.tile([128, KT, C], BF16)
        bqk_sb = persist.tile([128, FT], F32)
        bvb = persist.tile([128, C], F32)
        bpb = persist.tile([128, C], F32)
        cm = persist.tile([128, 128], BF16)
        warm_w = persist.tile([128, 128], BF16)
        warm_x = persist.tile([128, 512], BF16)
        xT = [persist.tile([128, KT, T], BF16, name=f"xT{b}", tag=f"xT{b}")
              for b in range(NB)]
        qkT = [[persist.tile([128, T], BF16, name=f"qkT{b}_{ft}",
                             tag=f"qkT{b}_{ft}") for ft in range(FT)]
               for b in range(NB)]
        vsb = [[persist.tile([128, H, DH + 1], BF16, name=f"v{b}_{rt}",
                             tag=f"v{b}_{rt}") for rt in range(JT)]
               for b in range(NB)]
        yT = [[[persist.tile([128, NI], BF16, name=f"yT{b}_{ib}_{hp}",
                             tag=f"yT{b}_{ib}_{hp}") for hp in range(HP)]
               for ib in range(NIB)] for b in range(NB)]

        # ---- preheat: warm the PE HAM clock gate while DMAs land ----
        nc.vector.memset(warm_w, 0.0)
        nc.vector.memset(warm_x, 0.0)
        for b in range(NB):
            for rt in range(JT):
                nc.vector.memset(vsb[b][rt][:, :, DH:DH + 1], 1.0)
        for _ in range(17):
            ps = ps01.tile([128, 512], F32, tag="ps", name="ps_warm")
            nc.tensor.matmul(ps, warm_w, warm_x, start=True, stop=True)

        # ---- initial loads. Full-row transfers only (wide DMA lines;
        # small-line column slices measured ~20x slower). Each queue
        # sustains only ~120GB/s, so the first-pass prerequisites (wqk +
        # x0) are spread across all three DMA-capable queues. ----
        def x_chunk(eng, b, kt, rb):
            eng.dma_start(
                out=xT[b][:, kt, rb * 512:(rb + 1) * 512],
                in_=xs_d[kt * 128:(kt + 1) * 128,
                         b * T + rb * 512:b * T + (rb + 1) * 512],
            )

        def wqk_row(eng, kt):
            eng.dma_start(out=wqk[:, kt, :],
                          in_=wqk_d[kt * 128:(kt + 1) * 128, :])

        def wv_row(eng, kt):
            eng.dma_start(out=wv[:, kt, :],
                          in_=wv_d[kt * 128:(kt + 1) * 128, :])

        nc.sync.dma_start(out=bqk_sb, in_=bqk_d.rearrange("(f p) -> p f", p=128))
        for qi, eng in enumerate((nc.sync, nc.scalar, nc.gpsimd)):
            wqk_row(eng, 2 * qi)
            x_chunk(eng, 0, qi, 0)
            x_chunk(eng, 0, 3 + qi, 0)
            wqk_row(eng, 2 * qi + 1)
            x_chunk(eng, 0, qi, 1)
            x_chunk(eng, 0, 3 + qi, 1)
            wv_row(eng, 2 * qi)
            wv_row(eng, 2 * qi + 1)
        nc.scalar.dma_start(
            out=bvb,
            in_=bass.AP(tensor=bv_d.tensor, offset=0,
                        ap=[[0, 128]] + list(bv_d.ap)),
        )
        nc.sync.dma_start(out=cm, in_=cm_d)
        for kt in range(KT):
            x_chunk(nc.sync, 1, kt, 0)
            x_chunk(nc.gpsimd, 1, kt, 1)
        for kt in range(KT):
            nc.scalar.dma_start(out=wp[:, kt, :],
                                in_=wp_d[kt * 128:(kt + 1) * 128, :])
        nc.scalar.dma_start(
            out=bpb,
            in_=bass.AP(tensor=bp_d.tensor, offset=0,
                        ap=[[0, 128]] + list(bp_d.ap)),
        )

        # ---- building blocks ----
        def chunk_A_qk(b, ft, rbs=None):
            for rb in (range(T // 512) if rbs is None else rbs):
                ps = ps01.tile([128, 512], F32, tag="ps", name="ps_qk")
                for kt in range(KT):
                    nc.tensor.matmul(
                        ps,
                        wqk[:, kt, ft * 128:(ft + 1) * 128],
                        xT[b][:, kt, rb * 512:(rb + 1) * 512],
                        start=(kt == 0),
                        stop=(kt == KT - 1),
                    )
                if b == 1:
                    # ACT epilogue: keeps the b=1 drains off the DVE, which
                    # carries the B-phase PSUM drains
                    nc.scalar.activation(
                        out=qkT[b][ft][:, rb * 512:(rb + 1) * 512], in_=ps,
                        func=mybir.ActivationFunctionType.Identity,
                        bias=bqk_sb[:, ft:ft + 1], scale=1.0,
                    )
                else:
                    nc.vector.tensor_scalar_add(
                        qkT[b][ft][:, rb * 512:(rb + 1) * 512], ps,
                        bqk_sb[:, ft:ft + 1],
                    )

        def chunk_A_v(b, rt):
            for g in range(2):
                ps = ps01.tile([128, 512], F32, tag="ps", name="ps_v")
                for kt in range(KT):
                    nc.tensor.matmul(
                        ps[:, 0:384],
                        xT[b][:, kt, rt * 128:(rt + 1) * 128],
                        wv[:, kt, g * 384:(g + 1) * 384],
                        start=(kt == 0),
                        stop=(kt == KT - 1),
                    )
                nc.vector.tensor_add(
                    vsb[b][rt][:, g * 6:(g + 1) * 6, 0:DH],
                    ps[:, 0:384].rearrange("p (h d) -> p h d", h=6),
                    bvb[:, g * 384:(g + 1) * 384].rearrange(
                        "p (h d) -> p h d", h=6
                    ),
                )

        pending_mul = []

        def flush_mul():
            while pending_mul:
                pending_mul.pop(0)()

        def unit_B(b, ib, hp, mids=()):
            """attention for batch b, i-block ib, head pair hp.

            Head A scores at s[:, cs:NI], head B shifted to s[:, NI:2NI-cs]
            so one exp covers a contiguous range. Diagonal j-tiles' PV
            matmuls are deferred to the unit tail so the GPSIMD mask
            multiply never stalls the PE. `mids` are filler chunks emitted
            mid-unit so the PE has work while ACT runs the exps."""
            mids = list(mids)
            pvA = psPV.tile([128, NI], F32, tag="pv", name="pvA")
            pvB = psPV.tile([128, NI], F32, tag="pv", name="pvB")
            njt = 4 * (ib + 1)
            mid_at = {1, njt - 2}  # after these j-tiles' exps
            deferred = []
            npv = 0

            def emit_pv(jt, cs, pT):
                nonlocal npv
                st = npv == 0
                sp = npv == 2 * njt - 2
                nc.tensor.matmul(
                    pvA[0:65, cs:],
                    vsb[b][jt][:, 2 * hp, :],
                    pT[:, cs:NI],
                    start=st, stop=sp,
                )
                nc.tensor.matmul(
                    pvB[0:65, cs:],
                    vsb[b][jt][:, 2 * hp + 1, :],
                    pT[:, NI:2 * NI - cs],
                    start=st, stop=sp,
                )
                npv += 2

            for jt in range(njt):
                cs = max(0, jt - 4 * ib) * 128
                s = psS.tile([128, 2 * NI], F32, tag="s", name="s")
                nc.tensor.matmul(
                    s[:, cs:NI],
                    qkT[b][HP + hp][0:64, jt * 128:jt * 128 + 128],
                    qkT[b][hp][0:64, ib * NI + cs:(ib + 1) * NI],
                    start=True, stop=True,
                    tile_position=(0, 0),
                )
                nc.tensor.matmul(
                    s[:, NI:2 * NI - cs],
                    qkT[b][HP + hp][64:128, jt * 128:jt * 128 + 128],
                    qkT[b][hp][64:128, ib * NI + cs:(ib + 1) * NI],
                    start=True, stop=True,
                    tile_position=(64, 0),
                )
                pT = pT_pool.tile([128, 2 * NI], BF16, tag="pT", name="pT")
                nc.scalar.activation(
                    out=pT[:, cs:2 * NI - cs], in_=s[:, cs:2 * NI - cs],
                    func=mybir.ActivationFunctionType.Exp,
                )
                if jt >= 4 * ib:  # diagonal subtile: mask, defer PV
                    # one strided multiply covers both heads' diagonal
                    # blocks (at cs and NI): halves the gpsimd op count
                    blk = bass.AP(
                        tensor=pT.tensor, offset=pT.offset + cs,
                        ap=[list(pT.ap[0]), [NI - cs, 2], [1, 128]],
                    )
                    cm2 = bass.AP(
                        tensor=cm.tensor, offset=cm.offset,
                        ap=[list(cm.ap[0]), [0, 2], [1, 128]],
                    )
                    nc.gpsimd.tensor_mul(blk, blk, cm2)
                    deferred.append((jt, cs, pT))
                else:
                    emit_pv(jt, cs, pT)
                if jt == 1:
                    flush_mul()
                if jt in mid_at and mids:
                    mids.pop(0)()
            for f in mids:
                f()
            for jt, cs, pT in deferred:
                emit_pv(jt, cs, pT)
            return pvA, pvB

        def unit_N(b, ib, hp, pvA, pvB, tail=False, chain_eng=None):
            """drain y^T; denominator rows staged bf16 at partitions {0,32},
            one strided gather to DRAM, flat [128,8] reciprocal, bounce back,
            two row-broadcasts; normalize with one [128,NI] multiply emitted
            one unit late so the DVE never waits on the DMA chain."""
            ce = chain_eng or nc.sync
            flush_mul()  # previous unit's multiply; its bc is long ready
            yt = yT[b][ib][hp]
            nc.vector.tensor_copy(yt[0:64, :], pvA[0:64, :])
            nc.vector.tensor_copy(yt[64:128, :], pvB[0:64, :])
            den = den_pool.tile([33, NI], BF16, tag="den", name="den")
            nc.vector.tensor_copy(den[0:1, :], pvA[64:65, :])
            nc.vector.tensor_copy(den[32:33, :], pvB[64:65, :])
            dd = dpool.tile([2, NI], BF16, tag="dd", name="dd")
            ce.dma_start(out=dd, in_=den[0:33:32, :])
            dci = dc_pool.tile([128, 8], BF16, tag="dci", name="dci")
            ce.dma_start(
                out=dci,
                in_=dd.rearrange("r c -> (r c)").rearrange("(p f) -> p f", p=128),
            )
            dco = dc_pool.tile([128, 8], F32, tag="dco", name="dco")
            nc.vector.reciprocal(dco, dci)
            dd2 = dpool.tile([128, 8], F32, tag="dd2", name="dd2")
            ce.dma_start(out=dd2, in_=dco)
            flat2 = dd2.rearrange("p f -> (p f)")
            bc = bc_pool.tile([128, NI], F32, tag="bc", name="bc")
            nc.gpsimd.dma_start(
                out=bc[0:64, :],
                in_=flat2[0:NI].rearrange("(p c) -> p c", p=1).to_broadcast((64, NI)),
            )
            nc.gpsimd.dma_start(
                out=bc[64:128, :],
                in_=flat2[NI:].rearrange("(p c) -> p c", p=1).to_broadcast((64, NI)),
            )
            pending_mul.append(lambda: nc.vector.tensor_mul(yt, yt, bc))
            if tail:
                flush_mul()

        def chunk_P(b, rt8, n_ct=KT, store_eng=None):
            """projection for batch b, one 128-row tile; first n_ct
            head-pair dim-blocks. Returns the open o_t when partial."""
            o_t = work.tile([128, C], F32, tag="o", name="o_t")
            for g, (c0, cw) in enumerate(((0, 512), (512, 256))):
                ps = ps01.tile([128, 512], F32, tag="ps", name="ps_o")
                for ct in range(n_ct):
                    nc.tensor.matmul(
                        ps[:, 0:cw],
                        yT[b][rt8 // 4][ct][:,
                                        (rt8 % 4) * 128:(rt8 % 4 + 1) * 128],
                        wp[:, ct, c0:c0 + cw],
                        start=(ct == 0),
                        stop=(ct == n_ct - 1),
                    )
                nc.vector.tensor_add(
                    o_t[:, c0:c0 + cw], ps[:, 0:cw], bpb[:, c0:c0 + cw]
                )
            if n_ct < KT:
                return o_t
            r0 = b * T + rt8 * 128
            (store_eng or nc.sync).dma_start(out=out_d[r0:r0 + 128, :], in_=o_t)

        def chunk_P_last(b, rt8, o_t, ct0, store_eng=None):
            """remaining dim-blocks of a partial projection + store."""
            for g, (c0, cw) in enumerate(((0, 512), (512, 256))):
                ps = ps01.tile([128, 512], F32, tag="ps", name="ps_o2")
                for i, ct in enumerate(range(ct0, KT)):
                    nc.tensor.matmul(
                        ps[:, 0:cw],
                        yT[b][rt8 // 4][ct][:,
                                        (rt8 % 4) * 128:(rt8 % 4 + 1) * 128],
                        wp[:, ct, c0:c0 + cw],
                        start=(i == 0),
                        stop=(ct == KT - 1),
                    )
                nc.vector.tensor_add(
                    o_t[:, c0:c0 + cw], o_t[:, c0:c0 + cw], ps[:, 0:cw]
                )
            r0 = b * T + rt8 * 128
            (store_eng or nc.sync).dma_start(out=out_d[r0:r0 + 128, :], in_=o_t)

        # ---- emission schedule ----
        # pre-phase: rb-interleaved; the first passes pace themselves on
        # the landing wqk/x rows (PE waits per-kt, keeping HAM active)
        for rb in range(2):
            for ft in (0, HP, 1, HP + 1):
                chunk_A_qk(0, ft, rbs=[rb])
        for rt in range(4):
            chunk_A_v(0, rt)

        def qk0(f):
            return lambda: chunk_A_qk(0, f)

        def qk1(f):
            return lambda: chunk_A_qk(1, f)

        def v0(rt):
            return lambda: chunk_A_v(0, rt)

        def v1(rt):
            return lambda: chunk_A_v(1, rt)

        def p0(r):
            return lambda: chunk_P(0, r)

        def p1(r):
            return lambda: chunk_P(1, r)

        # per-unit filler lists: each unit's qk prerequisites emitted at
        # least one unit ahead
        sched_b0 = [
            (0, 0, [v0(4), qk0(2), qk0(HP + 2)]),
            (0, 1, [v0(5), qk0(3), qk0(HP + 3)]),
            (0, 2, [v0(6), qk0(4), qk0(HP + 4)]),
            (0, 3, [v0(7), qk0(5), qk0(HP + 5)]),
            (0, 4, [qk1(0), qk1(HP)]),
            (0, 5, [qk1(1), qk1(HP + 1)]),
            (1, 0, [qk1(2), qk1(HP + 2), v1(0)]),
            (1, 1, [qk1(3), qk1(HP + 3), v1(1)]),
            (1, 2, [v1(2), v1(3)]),
            (1, 3, [v1(4), v1(5)]),
            (1, 4, [v1(6)]),
            (1, 5, [v1(7)]),
        ]
        for ib, hp, fills in sched_b0:
            pvA, pvB = unit_B(0, ib, hp, mids=fills[:2])
            unit_N(0, ib, hp, pvA, pvB)
            for f in fills[2:]:
                f()

        sched_b1 = [
            (1, 0, [p0(0)]),
            (1, 1, [qk1(4), p0(1)]),
            (1, 2, [qk1(HP + 4), p0(2)]),
            (1, 3, [qk1(5), p0(3)]),
            (1, 4, [qk1(HP + 5), p0(4)]),
            (1, 5, [p0(5), p0(6)]),
            (0, 0, [p0(7)]),
            (0, 1, [p1(4)]),
            (0, 2, [p1(5)]),
            (0, 3, [p1(6)]),
            (0, 4, [p1(7)]),
            (0, 5, []),
        ]
        for ib, hp, fills in sched_b1:
            pvA, pvB = unit_B(1, ib, hp, mids=fills[:2])
            # the last units' chains go via the idle scalar queue so they
            # don't pace each other on sync
            unit_N(1, ib, hp, pvA, pvB, tail=(ib == 0 and hp == HP - 1),
                   chain_eng=nc.scalar if (ib == 0 and hp >= 3) else None)
            for f in fills[2:]:
                f()
        # tail: main accumulations (head-pairs 0-4) cover the last unit's
        # normalize chain; only the small closing passes trail. Stores on
        # the idle scalar queue.
        tail_ot = [chunk_P(1, r, n_ct=KT - 1) for r in range(4)]
        for r in range(4):
            chunk_P_last(1, r, tail_ot[r], KT - 1, store_eng=nc.scalar)

    _split_excess_waits(nc)
    return nc


_PROG = None


def _get_program():
    global _PROG
    if _PROG is None:
        _PROG = _build_program()
    return _PROG


def kernel(x, attention_mask, W_attn, b_attn, W_proj, b_proj, **_unused):
    x = np.asarray(x, dtype=np.float32)
    W_attn = np.asarray(W_attn, dtype=np.float32)
    b_attn = np.asarray(b_attn, dtype=np.float32)
    W_proj = np.asarray(W_proj, dtype=np.float32)
    b_proj = np.asarray(b_proj, dtype=np.float32)

    bf = lambda a: np.ascontiguousarray(a).astype(ml_dtypes.bfloat16)
    scale = 1.0 / np.sqrt(DH)
    wqk = np.concatenate([W_attn[:, :C] * scale, W_attn[:, C:2 * C]], axis=1)
    bqk = np.concatenate([b_attn[:C] * scale, b_attn[C:2 * C]]).astype(np.float32)
    shared = {
        "wqk": bf(wqk),
        "wv": bf(W_attn[:, 2 * C:]),
        "wp": bf(W_proj),
        "bqk": bqk,
        "bv": b_attn[2 * C:].astype(np.float32),
        "bp": b_proj.astype(np.float32),
        # S^T tile is [j, i]; keep i >= j  ->  upper triangular incl. diagonal
        "cmask": bf(np.triu(np.ones((128, 128), np.float32))),
    }
    in_maps = []
    for c in range(N_CORES):
        xs = x[c * NB:(c + 1) * NB].reshape(R, C).T
        in_maps.append({"xs": bf(xs), **shared})

    nc = _get_program()
    globals()["_last_in_maps"] = in_maps
    try:
        res = run_bass_kernel_spmd(nc, in_maps, list(range(N_CORES)), trace=False)
    except Exception:
        # transient NRT device errors have been observed; retry once
        res = run_bass_kernel_spmd(nc, in_maps, list(range(N_CORES)), trace=False)
    out = np.empty((B, T, C), np.float32)
    for c in range(N_CORES):
        out[c * NB:(c + 1) * NB] = res.results[c]["out"].reshape(NB, T, C)
    return out


# revision 49
# speedup vs baseline: 1.0793x; 1.0793x over previous
"""Causal multi-head attention block (B=16, T=1024, C=768, H=12) on 8 Trainium2
NeuronCores.

Strategy: data-parallel over batch (2 batches per core, no collectives).
Per-core pipeline, all matmul operands bf16 with fp32 PSUM accumulation:
  A(b) qk^T = W_qk^T x^T (features on partitions); v = x W_v natural with a
       ones column per head (softmax denominators for free)
  B(b) per head-pair: S^T = K Q^T packed two heads per PE pass (K=64
       row-tiling, head B's output shifted down so exp covers one contiguous
       range), exp on ACT, causal mask via GPSIMD with the masked tiles'
       PV matmuls deferred to the unit tail (mask latency off the PE path)
  N(b) per unit: DVE reciprocal reads the denominator rows straight from
       PSUM, DRAM bounce broadcasts them, one [128,NI] DVE multiply
       normalizes both heads; the multiply is emitted one unit late so the
       DVE queue never waits on the broadcast DMA
  P(b) out = y W_proj + b_proj  (y^T is exactly the stationary operand layout)

Schedule: 8 preheat matmuls on zeroed tiles warm the PE HAM clock gate while
the first DMAs land; A(0) head, then B(0) with A(1) woven between units, then
B(1) (i-block 1 first) with P(0)/P(1) fillers; the final projections contract
the last-finished head-pair last so only a short normalize chain trails.
1/sqrt(64) folded into W_q host-side; biases applied via DVE epilogues.
"""

import sys

sys.path.insert(0, "/opt/trn_rl_repo")

import numpy as np
import ml_dtypes

import concourse.bass as bass
import concourse.tile as tile
from concourse import mybir
from concourse.bass_utils import run_bass_kernel_spmd

F32 = mybir.dt.float32
BF16 = mybir.dt.bfloat16

N_CORES = 8
B, T, C = 16, 1024, 768
H, DH = 12, 64
NB = B // N_CORES          # local batches per core (2)
R = NB * T                 # local rows (2048)
KT = C // 128              # contraction tiles (6)
FT = (2 * C) // 128        # qk feature tiles (12)
HP = H // 2                # head pairs (6)
NI = 512                   # i-block width
NIB = T // NI              # i-blocks per batch (2)
JT = T // 128              # j tiles per batch (8)


def _split_excess_waits(nc):
    """Walrus for this target accepts 1 semaphore wait per instruction
    (2 for EventSemaphore). Tile can emit more; split the excess onto
    same-engine nops placed immediately before the instruction."""
    nsplit = 0
    fn = nc.m.functions[0]
    cur = nc.cur_bb.bb if hasattr(nc.cur_bb, "bb") else nc.cur_bb
    for blk in fn.blocks:
        insts = list(blk.instructions)
        if not any(
            i.sync_info is not None
            and i.sync_info.on_wait
            and len(i.sync_info.on_wait)
            > (2 if type(i).__name__ == "InstEventSemaphore" else 1)
            for i in insts
        ):
            continue
        newlist, made = [], []
        for inst in insts:
            si = inst.sync_info
            maxw = 2 if type(inst).__name__ == "InstEventSemaphore" else 1
            if si is not None and si.on_wait and len(si.on_wait) > maxw:
                waits = list(si.on_wait)
                extra, keep = waits[:-maxw], waits[-maxw:]
                si.on_wait = keep
                for w in extra:
                    nop = nc.engines[inst.engine].nop()
                    nop.ins.sync_info = mybir.SyncInfo(on_wait=[w], on_update=[])
                    made.append(nop.ins)
                    newlist.append(nop.ins)
                    nsplit += 1
            newlist.append(inst)
        for m in made:
            if m in cur.instructions:
                cur.instructions.remove(m)
        blk.instructions[:] = newlist
    return nsplit


def _build_program():
    from contextlib import ExitStack

    nc = bass.Bass("TRN2", target_bir_lowering=False, debug=False)

    xs_d = nc.dram_tensor("xs", [C, R], BF16, kind="ExternalInput").ap()
    wqk_d = nc.dram_tensor("wqk", [C, 2 * C], BF16, kind="ExternalInput").ap()
    wv_d = nc.dram_tensor("wv", [C, C], BF16, kind="ExternalInput").ap()
    wp_d = nc.dram_tensor("wp", [C, C], BF16, kind="ExternalInput").ap()
    bqk_d = nc.dram_tensor("bqk", [2 * C], F32, kind="ExternalInput").ap()
    bv_d = nc.dram_tensor("bv", [C], F32, kind="ExternalInput").ap()
    bp_d = nc.dram_tensor("bp", [C], F32, kind="ExternalInput").ap()
    cm_d = nc.dram_tensor("cmask", [128, 128], BF16, kind="ExternalInput").ap()
    out_d = nc.dram_tensor("out", [R, C], F32, kind="ExternalOutput").ap()

    with tile.TileContext(nc) as tc, ExitStack() as ctx:
        persist = ctx.enter_context(tc.tile_pool(name="persist", bufs=1))
        work = ctx.enter_context(tc.tile_pool(name="work", bufs=4))
        pT_pool = ctx.enter_context(tc.tile_pool(name="pTp", bufs=5))
        bc_pool = ctx.enter_context(tc.tile_pool(name="bcp", bufs=3))
        den_pool = ctx.enter_context(tc.tile_pool(name="denp", bufs=3))
        dc_pool = ctx.enter_context(tc.tile_pool(name="dcp", bufs=4))
        ps01 = ctx.enter_context(tc.tile_pool(name="ps01", bufs=2, space="PSUM"))
        psS = ctx.enter_context(tc.tile_pool(name="psS", bufs=2, space="PSUM"))
        psPV = ctx.enter_context(tc.tile_pool(name="psPV", bufs=2, space="PSUM"))
        dpool = ctx.enter_context(tc.tile_pool(name="dpool", bufs=4, space="DRAM"))

        wqk = persist.tile([128, KT, 2 * C], BF16)
        wv = persist.tile([128, KT, C], BF16)
        wp = persist.tile([128, KT, C], BF16)
        bqk_sb = persist.tile([128, FT], F32)
        bvb = persist.tile([128, C], F32)
        bpb = persist.tile([128, C], F32)
        cm = persist.tile([128, 128], BF16)
        warm_w = persist.tile([128, 128], BF16)
        warm_x = persist.tile([128, 512], BF16)
        xT = [persist.tile([128, KT, T], BF16, name=f"xT{b}", tag=f"xT{b}")
              for b in range(NB)]
        qkT = [[persist.tile([128, T], BF16, name=f"qkT{b}_{ft}",
                             tag=f"qkT{b}_{ft}") for ft in range(FT)]
               for b in range(NB)]
        vsb = [[persist.tile([128, H, DH + 1], BF16, name=f"v{b}_{rt}",
                             tag=f"v{b}_{rt}") for rt in range(JT)]
               for b in range(NB)]
        yT = [[[persist.tile([128, NI], BF16, name=f"yT{b}_{ib}_{hp}",
                             tag=f"yT{b}_{ib}_{hp}") for hp in range(HP)]
               for ib in range(NIB)] for b in range(NB)]

        # ---- preheat: warm the PE HAM clock gate while DMAs land ----
        nc.vector.memset(warm_w, 0.0)
        nc.vector.memset(warm_x, 0.0)
        for b in range(NB):
            for rt in range(JT):
                nc.vector.memset(vsb[b][rt][:, :, DH:DH + 1], 1.0)
        for _ in range(12):
            ps = ps01.tile([128, 512], F32, tag="ps", name="ps_warm")
            nc.tensor.matmul(ps, warm_w, warm_x, start=True, stop=True)

        # ---- initial loads. Full-row transfers only (wide DMA lines;
        # small-line column slices measured ~20x slower). Each queue
        # sustains only ~120GB/s, so the first-pass prerequisites (wqk +
        # x0) are spread across all three DMA-capable queues. ----
        def x_chunk(eng, b, kt, rb):
            eng.dma_start(
                out=xT[b][:, kt, rb * 512:(rb + 1) * 512],
                in_=xs_d[kt * 128:(kt + 1) * 128,
                         b * T + rb * 512:b * T + (rb + 1) * 512],
            )

        def wqk_row(eng, kt):
            eng.dma_start(out=wqk[:, kt, :],
                          in_=wqk_d[kt * 128:(kt + 1) * 128, :])

        def wv_row(eng, kt):
            eng.dma_start(out=wv[:, kt, :],
                          in_=wv_d[kt * 128:(kt + 1) * 128, :])

        nc.sync.dma_start(out=bqk_sb, in_=bqk_d.rearrange("(f p) -> p f", p=128))
        for qi, eng in enumerate((nc.sync, nc.scalar, nc.gpsimd)):
            wqk_row(eng, 2 * qi)
            x_chunk(eng, 0, qi, 0)
            x_chunk(eng, 0, 3 + qi, 0)
            wqk_row(eng, 2 * qi + 1)
            x_chunk(eng, 0, qi, 1)
            x_chunk(eng, 0, 3 + qi, 1)
            wv_row(eng, 2 * qi)
            wv_row(eng, 2 * qi + 1)
        nc.scalar.dma_start(
            out=bvb,
            in_=bass.AP(tensor=bv_d.tensor, offset=0,
                        ap=[[0, 128]] + list(bv_d.ap)),
        )
        nc.sync.dma_start(out=cm, in_=cm_d)
        for kt in range(KT):
            x_chunk(nc.sync, 1, kt, 0)
            x_chunk(nc.gpsimd, 1, kt, 1)
        for kt in range(KT):
            nc.scalar.dma_start(out=wp[:, kt, :],
                                in_=wp_d[kt * 128:(kt + 1) * 128, :])
        nc.scalar.dma_start(
            out=bpb,
            in_=bass.AP(tensor=bp_d.tensor, offset=0,
                        ap=[[0, 128]] + list(bp_d.ap)),
        )

        # ---- building blocks ----
        def chunk_A_qk(b, ft, rbs=None):
            for rb in (range(T // 512) if rbs is None else rbs):
                ps = ps01.tile([128, 512], F32, tag="ps", name="ps_qk")
                for kt in range(KT):
                    nc.tensor.matmul(
                        ps,
                        wqk[:, kt, ft * 128:(ft + 1) * 128],
                        xT[b][:, kt, rb * 512:(rb + 1) * 512],
                        start=(kt == 0),
                        stop=(kt == KT - 1),
                    )
                if b == 1:
                    # ACT epilogue: keeps the b=1 drains off the DVE, which
                    # carries the B-phase PSUM drains
                    nc.scalar.activation(
                        out=qkT[b][ft][:, rb * 512:(rb + 1) * 512], in_=ps,
                        func=mybir.ActivationFunctionType.Identity,
                        bias=bqk_sb[:, ft:ft + 1], scale=1.0,
                    )
                else:
                    nc.vector.tensor_scalar_add(
                        qkT[b][ft][:, rb * 512:(rb + 1) * 512], ps,
                        bqk_sb[:, ft:ft + 1],
                    )

        def chunk_A_v(b, rt):
            for g in range(2):
                ps = ps01.tile([128, 512], F32, tag="ps", name="ps_v")
                for kt in range(KT):
                    nc.tensor.matmul(
                        ps[:, 0:384],
                        xT[b][:, kt, rt * 128:(rt + 1) * 128],
                        wv[:, kt, g * 384:(g + 1) * 384],
                        start=(kt == 0),
                        stop=(kt == KT - 1),
                    )
                nc.vector.tensor_add(
                    vsb[b][rt][:, g * 6:(g + 1) * 6, 0:DH],
                    ps[:, 0:384].rearrange("p (h d) -> p h d", h=6),
                    bvb[:, g * 384:(g + 1) * 384].rearrange(
                        "p (h d) -> p h d", h=6
                    ),
                )

        pending_mul = []

        def flush_mul():
            while pending_mul:
                pending_mul.pop(0)()

        def unit_B(b, ib, hp, mids=()):
            """attention for batch b, i-block ib, head pair hp.

            Head A scores at s[:, cs:NI], head B shifted to s[:, NI:2NI-cs]
            so one exp covers a contiguous range. Diagonal j-tiles' PV
            matmuls are deferred to the unit tail so the GPSIMD mask
            multiply never stalls the PE. `mids` are filler chunks emitted
            mid-unit so the PE has work while ACT runs the exps."""
            mids = list(mids)
            pvA = psPV.tile([128, NI], F32, tag="pv", name="pvA")
            pvB = psPV.tile([128, NI], F32, tag="pv", name="pvB")
            njt = 4 * (ib + 1)
            mid_at = {1, njt - 2}  # after these j-tiles' exps
            deferred = []
            npv = 0

            def emit_pv(jt, cs, pT):
                nonlocal npv
                st = npv == 0
                sp = npv == 2 * njt - 2
                nc.tensor.matmul(
                    pvA[0:65, cs:],
                    vsb[b][jt][:, 2 * hp, :],
                    pT[:, cs:NI],
                    start=st, stop=sp,
                )
                nc.tensor.matmul(
                    pvB[0:65, cs:],
                    vsb[b][jt][:, 2 * hp + 1, :],
                    pT[:, NI:2 * NI - cs],
                    start=st, stop=sp,
                )
                npv += 2

            for jt in range(njt):
                cs = max(0, jt - 4 * ib) * 128
                s = psS.tile([128, 2 * NI], F32, tag="s", name="s")
                nc.tensor.matmul(
                    s[:, cs:NI],
                    qkT[b][HP + hp][0:64, jt * 128:jt * 128 + 128],
                    qkT[b][hp][0:64, ib * NI + cs:(ib + 1) * NI],
                    start=True, stop=True,
                    tile_position=(0, 0),
                )
                nc.tensor.matmul(
                    s[:, NI:2 * NI - cs],
                    qkT[b][HP + hp][64:128, jt * 128:jt * 128 + 128],
                    qkT[b][hp][64:128, ib * NI + cs:(ib + 1) * NI],
                    start=True, stop=True,
                    tile_position=(64, 0),
                )
                pT = pT_pool.tile([128, 2 * NI], BF16, tag="pT", name="pT")
                nc.scalar.activation(
                    out=pT[:, cs:2 * NI - cs], in_=s[:, cs:2 * NI - cs],
                    func=mybir.ActivationFunctionType.Exp,
                )
                if jt >= 4 * ib:  # diagonal subtile: mask, defer PV
                    # one strided multiply covers both heads' diagonal
                    # blocks (at cs and NI): halves the gpsimd op count
                    blk = bass.AP(
                        tensor=pT.tensor, offset=pT.offset + cs,
                        ap=[list(pT.ap[0]), [NI - cs, 2], [1, 128]],
                    )
                    cm2 = bass.AP(
                        tensor=cm.tensor, offset=cm.offset,
                        ap=[list(cm.ap[0]), [0, 2], [1, 128]],
                    )
                    nc.gpsimd.tensor_mul(blk, blk, cm2)
                    deferred.append((jt, cs, pT))
                else:
                    emit_pv(jt, cs, pT)
                if jt == 1:
                    flush_mul()
                if jt in mid_at and mids:
                    mids.pop(0)()
            for f in mids:
                f()
            for jt, cs, pT in deferred:
                emit_pv(jt, cs, pT)
            return pvA, pvB

        def unit_N(b, ib, hp, pvA, pvB, tail=False, chain_eng=None):
            """drain y^T; denominator rows staged bf16 at partitions {0,32},
            one strided gather to DRAM, flat [128,8] reciprocal, bounce back,
            two row-broadcasts; normalize with one [128,NI] multiply emitted
            one unit late so the DVE never waits on the DMA chain."""
            ce = chain_eng or nc.sync
            flush_mul()  # previous unit's multiply; its bc is long ready
            yt = yT[b][ib][hp]
            nc.vector.tensor_copy(yt[0:64, :], pvA[0:64, :])
            nc.vector.tensor_copy(yt[64:128, :], pvB[0:64, :])
            den = den_pool.tile([33, NI], BF16, tag="den", name="den")
            nc.vector.tensor_copy(den[0:1, :], pvA[64:65, :])
            nc.vector.tensor_copy(den[32:33, :], pvB[64:65, :])
            dd = dpool.tile([2, NI], BF16, tag="dd", name="dd")
            ce.dma_start(out=dd, in_=den[0:33:32, :])
            dci = dc_pool.tile([128, 8], BF16, tag="dci", name="dci")
            ce.dma_start(
                out=dci,
                in_=dd.rearrange("r c -> (r c)").rearrange("(p f) -> p f", p=128),
            )
            dco = dc_pool.tile([128, 8], F32, tag="dco", name="dco")
            nc.vector.reciprocal(dco, dci)
            dd2 = dpool.tile([128, 8], F32, tag="dd2", name="dd2")
            ce.dma_start(out=dd2, in_=dco)
            flat2 = dd2.rearrange("p f -> (p f)")
            bc = bc_pool.tile([128, NI], F32, tag="bc", name="bc")
            nc.gpsimd.dma_start(
                out=bc[0:64, :],
                in_=flat2[0:NI].rearrange("(p c) -> p c", p=1).to_broadcast((64, NI)),
            )
            nc.gpsimd.dma_start(
                out=bc[64:128, :],
                in_=flat2[NI:].rearrange("(p c) -> p c", p=1).to_broadcast((64, NI)),
            )
            pending_mul.append(lambda: nc.vector.tensor_mul(yt, yt, bc))
            if tail:
                flush_mul()

        def chunk_P(b, rt8, n_ct=KT, store_eng=None):
            """projection for batch b, one 128-row tile; first n_ct
            head-pair dim-blocks. Returns the open o_t when partial."""
            o_t = work.tile([128, C], F32, tag="o", name="o_t")
            for g, (c0, cw) in enumerate(((0, 512), (512, 256))):
                ps = ps01.tile([128, 512], F32, tag="ps", name="ps_o")
                for ct in range(n_ct):
                    nc.tensor.matmul(
                        ps[:, 0:cw],
                        yT[b][rt8 // 4][ct][:,
                                        (rt8 % 4) * 128:(rt8 % 4 + 1) * 128],
                        wp[:, ct, c0:c0 + cw],
                        start=(ct == 0),
                        stop=(ct == n_ct - 1),
                    )
                nc.vector.tensor_add(
                    o_t[:, c0:c0 + cw], ps[:, 0:cw], bpb[:, c0:c0 + cw]
                )
            if n_ct < KT:
                return o_t
            r0 = b * T + rt8 * 128
            (store_eng or nc.sync).dma_start(out=out_d[r0:r0 + 128, :], in_=o_t)

        def chunk_P_last(b, rt8, o_t, ct0, store_eng=None):
            """remaining dim-blocks of a partial projection + store."""
            for g, (c0, cw) in enumerate(((0, 512), (512, 256))):
                ps = ps01.tile([128, 512], F32, tag="ps", name="ps_o2")
                for i, ct in enumerate(range(ct0, KT)):
                    nc.tensor.matmul(
                        ps[:, 0:cw],
                        yT[b][rt8 // 4][ct][:,
                                        (rt8 % 4) * 128:(rt8 % 4 + 1) * 128],
                        wp[:, ct, c0:c0 + cw],
                        start=(i == 0),
                        stop=(ct == KT - 1),
                    )
                nc.vector.tensor_add(
                    o_t[:, c0:c0 + cw], o_t[:, c0:c0 + cw], ps[:, 0:cw]
                )
            r0 = b * T + rt8 * 128
            (store_eng or nc.sync).dma_start(out=out_d[r0:r0 + 128, :], in_=o_t)

        # ---- emission schedule ----
        # pre-phase: rb-interleaved; the first passes pace themselves on
        # the landing wqk/x rows (PE waits per-kt, keeping HAM active)
        for rb in range(2):
            for ft in (0, HP, 1, HP + 1):
                chunk_A_qk(0, ft, rbs=[rb])
        for rt in range(4):
            chunk_A_v(0, rt)

        def qk0(f):
            return lambda: chunk_A_qk(0, f)

        def qk1(f):
            return lambda: chunk_A_qk(1, f)

        def v0(rt):
            return lambda: chunk_A_v(0, rt)

        def v1(rt):
            return lambda: chunk_A_v(1, rt)

        def p0(r):
            return lambda: chunk_P(0, r)

        def p1(r):
            return lambda: chunk_P(1, r)

        # per-unit filler lists: each unit's qk prerequisites emitted at
        # least one unit ahead
        sched_b0 = [
            (0, 0, [v0(4), qk0(2), qk0(HP + 2)]),
            (0, 1, [v0(5), qk0(3), qk0(HP + 3)]),
            (0, 2, [v0(6), qk0(4), qk0(HP + 4)]),
            (0, 3, [v0(7), qk0(5), qk0(HP + 5)]),
            (0, 4, [qk1(0), qk1(HP)]),
            (0, 5, [qk1(1), qk1(HP + 1)]),
            (1, 0, [qk1(2), qk1(HP + 2), v1(0)]),
            (1, 1, [qk1(3), qk1(HP + 3), v1(1)]),
            (1, 2, [v1(2), v1(3)]),
            (1, 3, [v1(4), v1(5)]),
            (1, 4, [v1(6)]),
            (1, 5, [v1(7)]),
        ]
        for ib, hp, fills in sched_b0:
            pvA, pvB = unit_B(0, ib, hp, mids=fills[:2])
            unit_N(0, ib, hp, pvA, pvB)
            for f in fills[2:]:
                f()

        sched_b1 = [
            (1, 0, [p0(0)]),
            (1, 1, [qk1(4), p0(1)]),
            (1, 2, [qk1(HP + 4), p0(2)]),
            (1, 3, [qk1(5), p0(3)]),
            (1, 4, [qk1(HP + 5), p0(4)]),
            (1, 5, [p0(5), p0(6)]),
            (0, 0, [p0(7)]),
            (0, 1, [p1(4)]),
            (0, 2, [p1(5)]),
            (0, 3, [p1(6)]),
            (0, 4, [p1(7)]),
            (0, 5, []),
        ]
        for ib, hp, fills in sched_b1:
            pvA, pvB = unit_B(1, ib, hp, mids=fills[:2])
            unit_N(1, ib, hp, pvA, pvB, tail=(ib == 0 and hp == HP - 1))
            for f in fills[2:]:
                f()
        # tail: main accumulations (head-pairs 0-4) cover the last unit's
        # normalize chain; only the small closing passes trail. Stores on
        # the idle scalar queue.
        tail_ot = [chunk_P(1, r, n_ct=KT - 1) for r in range(4)]
        for r in range(4):
            chunk_P_last(1, r, tail_ot[r], KT - 1, store_eng=nc.scalar)

    _split_excess_waits(nc)
    return nc


_PROG = None


def _get_program():
    global _PROG
    if _PROG is None:
        _PROG = _build_program()
    return _PROG


def kernel(x, attention_mask, W_attn, b_attn, W_proj, b_proj, **_unused):
    x = np.asarray(x, dtype=np.float32)
    W_attn = np.asarray(W_attn, dtype=np.float32)
    b_attn = np.asarray(b_attn, dtype=np.float32)
    W_proj = np.asarray(W_proj, dtype=np.float32)
    b_proj = np.asarray(b_proj, dtype=np.float32)

    bf = lambda a: np.ascontiguousarray(a).astype(ml_dtypes.bfloat16)
    scale = 1.0 / np.sqrt(DH)
    wqk = np.concatenate([W_attn[:, :C] * scale, W_attn[:, C:2 * C]], axis=1)
    bqk = np.concatenate([b_attn[:C] * scale, b_attn[C:2 * C]]).astype(np.float32)
    shared = {
        "wqk": bf(wqk),
        "wv": bf(W_attn[:, 2 * C:]),
        "wp": bf(W_proj),
        "bqk": bqk,
        "bv": b_attn[2 * C:].astype(np.float32),
        "bp": b_proj.astype(np.float32),
        # S^T tile is [j, i]; keep i >= j  ->  upper triangular incl. diagonal
        "cmask": bf(np.triu(np.ones((128, 128), np.float32))),
    }
    in_maps = []
    for c in range(N_CORES):
        xs = x[c * NB:(c + 1) * NB].reshape(R, C).T
        in_maps.append({"xs": bf(xs), **shared})

    nc = _get_program()
    globals()["_last_in_maps"] = in_maps
    try:
        res = run_bass_kernel_spmd(nc, in_maps, list(range(N_CORES)), trace=False)
    except Exception:
        # transient NRT device errors have been observed; retry once
        res = run_bass_kernel_spmd(nc, in_maps, list(range(N_CORES)), trace=False)
    out = np.empty((B, T, C), np.float32)
    for c in range(N_CORES):
        out[c * NB:(c + 1) * NB] = res.results[c]["out"].reshape(NB, T, C)
    return out
